# revision 40
# baseline (speedup 1.0000x reference)
"""Trainium2 Bass kernel for nn_DecoderLSTMAttention.

Data-parallel over batch: 8 cores x 8 samples. No collectives.
Key restructurings (validated in numpy mock, bf16 rel-err ~4e-3):
  - pf := features @ W_ih[EMB:]  precomputed once per core -> per-step
    context-gates become (alpha_bd @ pf), contraction over N=196 not ENC.
  - context never materialized; xe := emb[captions] @ W_ih[:EMB] + biases
    precomputed (host) and accumulated into gates PSUM via identity matmul.
  - fcn deferred: H collected transposed, one [152,512]@[512,10000] at end.
  - attention kept transposed [ATT, (b,n)]; sigma via tanh trick so the only
    ACT tables used are {Tanh, Exp} (one table set).
  - gates transposed per step via DMA-xbar ([16,2048]bf16 -> [128,16,16]).
"""
import os
import sys
import numpy as np
import ml_dtypes

sys.path.insert(0, "/opt/trn_rl_repo")

BF = ml_dtypes.bfloat16
B, N, ENC = 64, 196, 2048
DEC, ATT, EMB, V, T = 512, 512, 512, 10000, 20
TS = T - 1           # 19 recurrent steps
NCORES = 8
BL = B // NCORES     # 8 samples per core
BN = BL * N          # 1568

_cache = {}


def _build_program():
    import concourse.bass as bass
    import concourse.tile as tile
    from concourse import bacc, mybir

    f32 = mybir.dt.float32
    bf = mybir.dt.bfloat16
    AF = mybir.ActivationFunctionType
    ALU = mybir.AluOpType

    nc = bacc.Bacc()
    # ---- DRAM I/O (host packs everything into SBUF-friendly layouts) ----
    dIn = lambda n_, s, d=bf: nc.dram_tensor(n_, s, d, kind="ExternalInput")
    featT = dIn("featT", [128, 16, BN])          # features^T: [p,kc,bn] k=kc*128+p
    uw = dIn("uw", [128, 16, ATT])               # Uw k-chunked
    wc = dIn("wc", [128, 16, 2048])              # W_ih[EMB:] (perm cols), k-chunked
    whh = dIn("whh", [128, 4, 2048])             # W_hh (perm cols), k-chunked
    ww = dIn("ww", [128, 4, ATT])                # Ww k-chunked
    awrep = dIn("awrep", [128, 4, BL])           # Aw replicated 8 cols, k-chunked
    fcnw = dIn("fcnw", [128, 4, V])              # fcn_w k-chunked
    fcnb = dIn("fcnb", [1, V])
    ones = dIn("ones", [1, TS * BL])
    xe = dIn("xe", [TS, BL, 2048])               # embeds@W_e + b_ih+b_hh (perm)
    ubwb = dIn("ubwb", [1, ATT])                 # (Ub+Wb) row, bf16
    h0 = dIn("h0", [128, 4, BL], f32)            # h0^T packed
    c0 = dIn("c0", [128, 4, BL], f32)
    id8 = dIn("id8", [BL, BL])                   # identity bf16
    preds_o = nc.dram_tensor("preds", [TS * BL, V], f32, kind="ExternalOutput")
    alphas_o = nc.dram_tensor("alphas", [BL, TS, N], f32, kind="ExternalOutput")

    NCH_S = [(0, 512), (512, 512), (1024, 512), (1536, 32)]  # scores n-chunks

    with tile.TileContext(nc) as tc:
        from contextlib import ExitStack
        with ExitStack() as ctx:
            P = ctx.enter_context(tc.tile_pool(name="persist", bufs=1))
            pf_sb = P.tile([128, 16, 2048], bf)     # per-b padded (b,half) chunks
            u_sb = P.tile([128, 4, BN], bf)         # u_hsT + (Ub+Wb)
            whh_sb = P.tile([128, 4, 2048], bf)
            ww_sb = P.tile([128, 4, ATT], bf)
            aw_sb = P.tile([128, 4, BL], bf)
            ubwb_sb = P.tile([1, ATT], bf)
            ones_sb = P.tile([1, TS * BL], bf)
            id8_sb = P.tile([BL, BL], bf)
            hT = P.tile([128, 4, BL], f32)
            cT = P.tile([128, 4, BL], f32)
            hTb = P.tile([128, 4, BL], bf)
            HT = P.tile([128, 4, TS * BL], bf)      # h'T history for end-fcn
            ebf = P.tile([BL, 256], bf)             # alpha padded, bf16
            e_bd = P.tile([128, 16, BL], bf)        # block-diag alpha^T (lhsT)
            ga = P.tile([16, 2048], bf)             # activated gates (rows 0..7)
            gaT = P.tile([128, 16, 16], bf)         # transposed activated gates
            sg = P.tile([128, 12, BL], f32)         # sigmoid(i,f,o)
            gf = P.tile([128, 4, BL], f32)          # tanh(g) upcast
            th = P.tile([128, 4, BL], f32)          # tanh(c)
            whT = P.tile([128, 4, BL], bf)

            # one-time zero-init + param loads
            nc.gpsimd.memset(pf_sb[:], 0.0)
            nc.vector.memset(ebf[:], 0.0)
            nc.gpsimd.memset(e_bd[:], 0.0)
            nc.gpsimd.memset(ga[:], 0.0)
            nc.sync.dma_start(whh_sb[:], whh[:])
            nc.sync.dma_start(ww_sb[:], ww[:])
            nc.sync.dma_start(aw_sb[:], awrep[:])
            nc.sync.dma_start(ubwb_sb[:], ubwb[:])
            nc.sync.dma_start(ones_sb[:], ones[:])
            nc.sync.dma_start(id8_sb[:], id8[:])
            nc.sync.dma_start(hT[:], h0[:])
            nc.sync.dma_start(cT[:], c0[:])
            nc.vector.tensor_copy(hTb[:], hT[:])

            # ================= precompute =================
            with tc.tile_pool(name="featp", bufs=1) as fp:
                ft = fp.tile([128, 16, BN], bf)
                nc.sync.dma_start(ft[:], featT[:])

                # ---- u_hsT = (feat @ Uw)^T + (Ub+Wb), flipped layout ----
                with tc.tile_pool(name="uwp", bufs=1) as uwp, \
                     tc.tile_pool(name="ups", bufs=2, space="PSUM") as ups:
                    uw_sb = uwp.tile([128, 16, ATT], bf)
                    nc.sync.dma_start(uw_sb[:], uw[:])
                    for m in range(4):
                        for (n0, nsz) in NCH_S:
                            ps = ups.tile([128, 512], f32, tag="ups")
                            for k in range(16):
                                nc.tensor.matmul(
                                    ps[:, :nsz],
                                    uw_sb[:, k, m * 128:(m + 1) * 128],
                                    ft[:, k, n0:n0 + nsz],
                                    start=(k == 0), stop=(k == 15))
                            nc.vector.tensor_copy(
                                u_sb[:, m, n0:n0 + nsz], ps[:, :nsz])

                # ---- pf = feat @ W_c, per-b padded [*,2b+half,*] ----
                with tc.tile_pool(name="wcp", bufs=2) as wcp, \
                     tc.tile_pool(name="pps", bufs=2, space="PSUM") as pps:
                    for nch in range(4):
                        wct = wcp.tile([128, 16, 512], bf, tag="wc")
                        nc.sync.dma_start(
                            wct[:], wc[:, :, nch * 512:(nch + 1) * 512])
                        for b in range(BL):
                            for half in range(2):
                                msz = 128 if half == 0 else N - 128  # 68
                                c0_ = b * N + half * 128
                                ps = pps.tile([128, 512], f32, tag="pps")
                                for k in range(16):
                                    nc.tensor.matmul(
                                        ps[:msz, :],
                                        ft[:, k, c0_:c0_ + msz],
                                        wct[:, k, :],
                                        start=(k == 0), stop=(k == 15))
                                eng = nc.vector if (b + half) % 2 else nc.scalar
                                dst = pf_sb[:msz, 2 * b + half,
                                            nch * 512:(nch + 1) * 512]
                                if eng is nc.vector:
                                    nc.vector.tensor_copy(dst, ps[:msz, :])
                                else:
                                    nc.scalar.copy(dst, ps[:msz, :])

            # ================= recurrent steps =================
            with tc.tile_pool(name="xep", bufs=2) as xep, \
                 tc.tile_pool(name="attp", bufs=3) as attp, \
                 tc.tile_pool(name="sfx", bufs=2) as sfx, \
                 tc.tile_pool(name="fwp", bufs=3) as fwp, \
                 tc.tile_pool(name="gps", bufs=1, space="PSUM") as gps, \
                 tc.tile_pool(name="wps", bufs=1, space="PSUM") as wps:

                for t in range(TS):
                    # -- WhT = Ww^T @ h (flipped, lands [att,b]) --
                    wht_ps = wps.tile([128, 4, BL], f32, tag="wht")
                    for m in range(4):
                        for k in range(4):
                            nc.tensor.matmul(
                                wht_ps[:, m, :],
                                ww_sb[:, k, m * 128:(m + 1) * 128],
                                hTb[:, k, :],
                                start=(k == 0), stop=False)
                        # + (Ub+Wb) broadcast over batch: ubwb^T @ ones_row
                        nc.tensor.matmul(
                            wht_ps[:, m, :],
                            ubwb_sb[:, m * 128:(m + 1) * 128],
                            ones_sb[:, 0:BL],
                            start=False, stop=True)
                    nc.vector.tensor_copy(whT[:], wht_ps[:])

                    # -- attention: att = tanh(u + Wh), scores = Aw . att --
                    sc_ps = gps.tile([BL, BN], f32, tag="gbank")
                    for a in range(4):
                        ap_ = attp.tile([128, BL, N], bf, tag="apre")
                        nc.vector.tensor_tensor(
                            ap_[:],
                            u_sb[:, a, :].rearrange("p (b n) -> p b n", b=BL),
                            whT[:, a, :].rearrange("p (b one) -> p b one", one=1)
                                .broadcast_to([128, BL, N]),
                            ALU.add)
                        at_ = attp.tile([128, BN], bf, tag="atanh")
                        nc.scalar.activation(at_[:], ap_[:].rearrange("p b n -> p (b n)"),
                                             AF.Tanh)
                        for (n0, nsz) in NCH_S:
                            nc.tensor.matmul(
                                sc_ps[:, n0:n0 + nsz],
                                aw_sb[:, a, :],
                                at_[:, n0:n0 + nsz],
                                start=(a == 0), stop=(a == 3))

                    # -- softmax (rows of sc_ps identical; one strided DMA
                    # re-partitions row 0 into [8, 196]) --
                    e_full = sfx.tile([1, BN], f32, tag="efull")
                    nc.scalar.activation(e_full[:], sc_ps[0:1, :], AF.Exp)
                    e_cmp = sfx.tile([BL, N], f32, tag="ecmp")
                    e_src = bass.AP(e_full.tensor, e_full.offset,
                                    [[1, 1], [N, BL], [1, N]])
                    nc.sync.dma_start(e_cmp[:], e_src)
                    ssum = sfx.tile([BL, 1], f32, tag="ssum")
                    nc.vector.reduce_sum(ssum[:], e_cmp[:],
                                         axis=mybir.AxisListType.X)
                    srcp = sfx.tile([BL, 1], f32, tag="srcp")
                    nc.vector.reciprocal(srcp[:], ssum[:])
                    alpha = sfx.tile([BL, N], f32, tag="alpha")
                    nc.vector.tensor_tensor(alpha[:], e_cmp[:],
                                            srcp[:, 0:1].broadcast_to([BL, N]),
                                            ALU.mult)
                    nc.sync.dma_start(alphas_o[:, t, :], alpha[:])
                    nc.vector.tensor_copy(ebf[:, :N], alpha[:])
                    # alpha^T via PE transpose (identity=id8), then scatter
                    # into the block-diagonal lhsT positions (one strided copy:
                    # e_bd[p, 2b+h, b] <- et_ps[p, h, b])
                    et_ps = wps.tile([128, 2, BL], bf, tag="etp")
                    nc.tensor.transpose(et_ps[:, 0, :], ebf[:, 0:128], id8_sb[:])
                    nc.tensor.transpose(et_ps[:, 1, :], ebf[:, 128:256], id8_sb[:])
                    ebd_diag = bass.AP(e_bd.tensor, e_bd.offset,
                                       [[128, 128], [17, BL], [BL, 2]])
                    nc.vector.tensor_copy(
                        ebd_diag, et_ps[:].rearrange("p h b -> p b h"))

                    # -- gates PSUM: h@W_hh + xe_t + per-b alpha@pf --
                    xet = xep.tile([BL, 2048], bf, tag="xe")
                    nc.sync.dma_start(xet[:], xe[t])
                    g_ps = gps.tile([BL, 2048], f32, tag="gbank")
                    for q in range(4):
                        nsl = slice(q * 512, (q + 1) * 512)
                        for k in range(4):
                            nc.tensor.matmul(g_ps[:, nsl], hTb[:, k, :],
                                             whh_sb[:, k, nsl],
                                             start=(k == 0), stop=False)
                        nc.tensor.matmul(g_ps[:, nsl], id8_sb[:], xet[:, nsl],
                                         start=False, stop=False)
                        for k in range(16):
                            nc.tensor.matmul(g_ps[:, nsl], e_bd[:, k, :],
                                             pf_sb[:, k, nsl],
                                             start=False, stop=(k == 15))

                    # -- activations on [8,2048], then xbar transpose --
                    nc.scalar.activation(ga[0:BL, 0:1536], g_ps[:, 0:1536],
                                         AF.Tanh, scale=0.5)
                    nc.scalar.activation(ga[0:BL, 1536:2048], g_ps[:, 1536:2048],
                                         AF.Tanh)
                    nc.sync.dma_start_transpose(gaT[:], ga[:])

                    # -- LSTM cell on [128, *, 8] --
                    nc.vector.tensor_scalar(sg[:], gaT[:, 0:12, 0:BL],
                                            0.5, 0.5, ALU.mult, ALU.add)
                    nc.vector.tensor_copy(gf[:], gaT[:, 12:16, 0:BL])
                    nc.vector.tensor_tensor(th[:], sg[:, 4:8, :], cT[:], ALU.mult)
                    nc.vector.tensor_tensor(gf[:], sg[:, 0:4, :], gf[:], ALU.mult)
                    nc.vector.tensor_tensor(cT[:], th[:], gf[:], ALU.add)
                    nc.scalar.activation(th[:], cT[:], AF.Tanh)
                    nc.vector.tensor_tensor(hT[:], sg[:, 8:12, :], th[:], ALU.mult)
                    nc.vector.tensor_copy(hTb[:], hT[:])
                    nc.vector.tensor_copy(HT[:, :, t * BL:(t + 1) * BL], hT[:])

            # ================= tail: preds = H @ fcn_w + fcn_b =================
            with tc.tile_pool(name="fwp2", bufs=3) as fwp2, \
                 tc.tile_pool(name="fbp", bufs=2) as fbp, \
                 tc.tile_pool(name="pout", bufs=3) as pout, \
                 tc.tile_pool(name="tps", bufs=2, space="PSUM") as tps:
                fbt_all = fbp.tile([1, V], bf, tag="fb")
                nc.sync.dma_start(fbt_all[:], fcnb[:])
                for nch in range(20):
                    nsl = slice(nch * 500, (nch + 1) * 500)
                    fwt = fwp2.tile([128, 4, 500], bf, tag="fw")
                    nc.sync.dma_start(fwt[:], fcnw[:, :, nsl])
                    fbt = fbt_all[:, nsl]
                    for m in range(2):
                        msz = 128 if m == 0 else TS * BL - 128  # 24
                        ps = tps.tile([128, 500], f32, tag="tps")
                        for k in range(4):
                            nc.tensor.matmul(
                                ps[:msz, :],
                                HT[:, k, m * 128:m * 128 + msz],
                                fwt[:, k, :],
                                start=(k == 0), stop=False)
                        nc.tensor.matmul(
                            ps[:msz, :], ones_sb[:, m * 128:m * 128 + msz],
                            fbt, start=False, stop=True)
                        po = pout.tile([128, 500], f32, tag="po")
                        eng = nc.vector if nch % 2 else nc.scalar
                        if eng is nc.vector:
                            nc.vector.tensor_copy(po[:msz, :], ps[:msz, :])
                        else:
                            nc.scalar.copy(po[:msz, :], ps[:msz, :])
                        nc.sync.dma_start(
                            preds_o[m * 128:m * 128 + msz, nsl], po[:msz, :])
    nc.finalize()
    return nc


def _host_prep(inputs):
    """Split/pack/cast all inputs per core. Returns list of in_maps."""
    f = {k: np.asarray(v) for k, v in inputs.items()}
    feat = f["features"].astype(np.float32)         # [64,196,2048]
    caps = np.asarray(f["captions"]).astype(np.int64)
    emb = f["emb"].astype(np.float32)
    perm = np.concatenate([np.arange(0, 512), np.arange(512, 1024),
                           np.arange(1536, 2048), np.arange(1024, 1536)])
    W_ihP = f["W_ih"][:, perm].astype(np.float32)
    W_hhP = f["W_hh"][:, perm].astype(np.float32)
    bP = (f["b_ih"] + f["b_hh"])[perm].astype(np.float32)
    W_e, W_c = W_ihP[:EMB], W_ihP[EMB:]

    def kchunk(w):  # [K, M] -> [128, K/128, M]
        K, M = w.shape
        return np.ascontiguousarray(
            w.reshape(K // 128, 128, M).transpose(1, 0, 2))

    wc_p = kchunk(W_c).astype(BF)
    whh_p = kchunk(W_hhP).astype(BF)
    ww_p = kchunk(f["Ww"].astype(np.float32)).astype(BF)
    uw_p = kchunk(f["Uw"].astype(np.float32)).astype(BF)
    aw_p = kchunk(np.repeat(f["Aw"].astype(np.float32), BL, 1)).astype(BF)
    fw_p = kchunk(f["fcn_w"].astype(np.float32)).astype(BF)
    fb_p = f["fcn_b"].astype(np.float32).reshape(1, V).astype(BF)
    ones_p = np.ones((1, TS * BL), np.float32).astype(BF)
    ubwb_p = (f["Ub"] + f["Wb"]).astype(np.float32).reshape(1, ATT).astype(BF)
    id8_p = np.eye(BL, dtype=np.float32).astype(BF)

    in_maps = []
    for c in range(NCORES):
        sl = slice(c * BL, (c + 1) * BL)
        fc = feat[sl].reshape(BN, ENC)              # [1568, 2048]
        featT_p = np.ascontiguousarray(
            fc.T.reshape(16, 128, BN).transpose(1, 0, 2)).astype(BF)
        embeds = emb[caps[sl, :TS]]                  # [8,19,512]
        xe_p = (np.einsum("bte,eg->tbg", embeds, W_e) + bP).astype(BF)
        mf = feat[sl].mean(1)                        # [8, 2048]
        h0_ = (mf @ f["ihw"] + f["ihb"]).astype(np.float32)   # [8,512]
        c0_ = (mf @ f["icw"] + f["icb"]).astype(np.float32)
        pack_state = lambda x: np.ascontiguousarray(
            x.T.reshape(4, 128, BL).transpose(1, 0, 2))
        in_maps.append({
            "featT": featT_p, "uw": uw_p, "wc": wc_p, "whh": whh_p,
            "ww": ww_p, "awrep": aw_p, "fcnw": fw_p, "fcnb": fb_p,
            "xe": np.ascontiguousarray(xe_p), "ubwb": ubwb_p, "ones": ones_p,
            "h0": pack_state(h0_), "c0": pack_state(c0_), "id8": id8_p,
        })
    return in_maps


def kernel(**inputs):
    if "prog" not in _cache:
        _cache["prog"] = _build_program()
    nc = _cache["prog"]
    in_maps = _host_prep(inputs)

    from concourse.bass_utils import run_bass_kernel_spmd
    res = run_bass_kernel_spmd(nc, in_maps, list(range(NCORES)))
    return _collect(res)


def _collect(res):
    preds = np.zeros((B, TS, V), np.float32)
    alphas = np.zeros((B, TS, N), np.float32)
    for c in range(NCORES):
        r = res.results[c]
        preds[c * BL:(c + 1) * BL] = (
            r["preds"].reshape(TS, BL, V).transpose(1, 0, 2))
        alphas[c * BL:(c + 1) * BL] = r["alphas"]
    return preds, alphas


def _install_ntff_hook():
    """Re-create the trn_boot NTFF hook (antenv.axon_hooks was absent)."""
    import ctypes
    import contextlib
    import types
    try:
        from antenv import axon_hooks
    except ImportError:
        import antenv
        axon_hooks = types.ModuleType("antenv.axon_hooks")
        axon_hooks._HOOK = None
        def _set(h, _m=axon_hooks):
            _m._HOOK = h
        def _get(_m=axon_hooks):
            return _m._HOOK
        axon_hooks.set_axon_ntff_profile_hook = _set
        axon_hooks.get_axon_ntff_profile_hook = _get
        sys.modules["antenv.axon_hooks"] = axon_hooks
        antenv.axon_hooks = axon_hooks
    if axon_hooks.get_axon_ntff_profile_hook() is not None:
        return
    lib = ctypes.CDLL("/opt/axon/libaxon_pjrt.so")
    if not hasattr(lib, "axon_start_nrt_profile"):
        return
    lib.axon_start_nrt_profile.argtypes = [ctypes.POINTER(ctypes.c_int64),
                                           ctypes.c_size_t]
    lib.axon_start_nrt_profile.restype = ctypes.c_int64
    lib.axon_stop_nrt_profile.argtypes = [ctypes.c_char_p]
    lib.axon_stop_nrt_profile.restype = ctypes.c_int64

    @contextlib.contextmanager
    def _hook(output_dir, device_ids):
        import jax
        jax.devices()
        if device_ids:
            ids = (ctypes.c_int64 * len(device_ids))(*device_ids)
            rc = lib.axon_start_nrt_profile(ids, len(device_ids))
        else:
            rc = lib.axon_start_nrt_profile(None, 0)
        if rc != 0:
            raise RuntimeError(f"axon_start_nrt_profile rc={rc}")
        try:
            yield
        finally:
            n = lib.axon_stop_nrt_profile(str(output_dir).encode())
            print(f"profile: {n} file(s) written to {output_dir}")

    axon_hooks.set_axon_ntff_profile_hook(_hook)


def profile_exec_ns(inputs, tmpdir=None):
    """Run with NTFF tracing; returns exec_time_ns (and leaves trace files
    in tmpdir for perfetto analysis)."""
    _install_ntff_hook()
    if "prog" not in _cache:
        _cache["prog"] = _build_program()
    nc = _cache["prog"]
    in_maps = _host_prep(inputs)
    from concourse.bass_utils import run_bass_kernel_spmd
    if tmpdir is None:
        tmpdir = "/tmp/ktrace"
        os.makedirs(tmpdir, exist_ok=True)
    res = run_bass_kernel_spmd(nc, in_maps, list(range(NCORES)),
                               trace=True, tmpdir=tmpdir)
    print("trace dir:", tmpdir)
    print("exec_time_ns:", res.exec_time_ns,
          "mean:", res.mean_exec_time_ns)
    return res.exec_time_ns
